# revision 17
# baseline (speedup 1.0000x reference)
"""Trainium2 Bass kernel: SAM2-style full self-attention over a 3D volume.

Computes  out = x + gamma * softmax((x Wq + bq)(x Wk + bk)^T / sqrt(d)) (x Wv + bv)
for x of shape [1, 20, 20, 20, 256]  (N = 8000 tokens, C = 256, d = 32).

Sharding: sequence-parallel over query rows. Core c owns output rows
[c*1000, (c+1)*1000). Every core recomputes the full K / V projections from
the complete x (only ~8 MB), so no collectives are needed.

Per-core dataflow (bf16 matmuls, fp32 PSUM accumulation, fp32 residual):
  - host pre-transposes x -> xT [C, N]; the channel contraction is the SBUF
    partition dim everywhere, so there are no on-device transposes.
  - qT/kT are replicated across four 32-partition blocks (host tiles Wq/Wk to
    [C, 128]) so pairs of K=32 score matmuls pack into disjoint PE row groups
    (tile_position) and run concurrently.
  - v [N, 257]: col 256 memset to 1.0 - the ones column makes attn@v also
    produce the softmax denominators for free.
  - scores are computed in groups of two 128-key chunks into one 2-bank PSUM
    tile; a single Exp activation per group (scale 1/sqrt(d) folded in; no
    max-subtraction needed: |scores|/sqrt(d) < ~6 so fp32 exp cannot
    overflow) writes bf16 S_exp^T into a per-block SBUF buffer.
  - attn@v runs j-inner per 128-query sub-tile: long same-PSUM-bank
    accumulation chains, K=128, N=257.
  - epilogue: out = (attended[:, :256] * gamma / attended[:, 256]) + x.
"""

import math

import numpy as np
import ml_dtypes

import concourse.bacc as bacc
import concourse.mybir as mybir
import concourse.tile as tile
from concourse.bass_utils import run_bass_kernel_spmd

BF16 = mybir.dt.bfloat16
F32 = mybir.dt.float32
AF = mybir.ActivationFunctionType
ALU = mybir.AluOpType

N_CORES = 8
FULL_N = 8000
FULL_C = 256

GROUP = 2     # score j-chunks per PSUM tile / Exp call (row-group packing)
BLOCK_G = 4   # groups per attn block (8 j-chunks): same-bank chain length
USE_FP8 = False  # fp8e4m3 S_exp/v with DoubleRow attn@v (2 key chunks per matmul)
FP8 = mybir.dt.float8e4
VPAD = 272    # v chunk stride (16B-aligned for DoubleRow APs)


def _slices(total, width):
    out = []
    o = 0
    while o < total:
        w = min(width, total - o)
        out.append((o, w))
        o += w
    return out


def build(n, rows, c, d, has_bv, has_bqk):
    assert c % 128 == 0
    assert 4 * d == 128
    kc_n = c // 128
    jchunks = _slices(n, 128)
    n_j = len(jchunks)
    groups = [list(range(g, min(g + GROUP, n_j))) for g in range(0, n_j, GROUP)]
    scale = 1.0 / math.sqrt(d)

    nc = bacc.Bacc("TRN2", target_bir_lowering=False, debug=False,
                   enable_asserts=False, num_devices=N_CORES)

    xT = nc.dram_tensor("xT", [c, n], BF16, kind="ExternalInput").ap()
    xTq = nc.dram_tensor("xTq", [c, rows], BF16, kind="ExternalInput").ap()
    xres = nc.dram_tensor("xres", [rows, c], F32, kind="ExternalInput").ap()
    wq = nc.dram_tensor("wq", [c, 4 * d], BF16, kind="ExternalInput").ap()
    wk = nc.dram_tensor("wk", [c, 4 * d], BF16, kind="ExternalInput").ap()
    wv = nc.dram_tensor("wv", [c, c], BF16, kind="ExternalInput").ap()
    bq = bk = bv = None
    if has_bqk:
        bq = nc.dram_tensor("bq", [128, 1], F32, kind="ExternalInput").ap()
        bk = nc.dram_tensor("bk", [128, 1], F32, kind="ExternalInput").ap()
    if has_bv:
        bv = nc.dram_tensor("bv", [1, c], F32, kind="ExternalInput").ap()
    gscale = nc.dram_tensor("gscale", [128, 1], F32, kind="ExternalInput").ap()
    out = nc.dram_tensor("out", [rows, c], F32, kind="ExternalOutput").ap()

    with tile.TileContext(nc) as tc:
        with (
            tc.tile_pool(name="consts", bufs=1) as consts,
            tc.tile_pool(name="sexp", bufs=3) as sep,
            tc.tile_pool(name="small", bufs=4) as smallp,
            tc.tile_pool(name="xrp", bufs=3) as xrp,
            tc.tile_pool(name="otp", bufs=3) as otp,
            tc.tile_pool(name="psp", bufs=2, space="PSUM") as psp,
        ):
            # ---- small inputs first: weights, biases, per-core query cols ----
            wq_sb = consts.tile([128, kc_n, 4 * d], BF16, name="wq_sb")
            wk_sb = consts.tile([128, kc_n, 4 * d], BF16, name="wk_sb")
            wv_sb = consts.tile([128, kc_n, c], BF16, name="wv_sb")
            for kc in range(kc_n):
                nc.scalar.dma_start(out=wq_sb[:, kc, :], in_=wq[kc * 128:(kc + 1) * 128, :])
                nc.scalar.dma_start(out=wk_sb[:, kc, :], in_=wk[kc * 128:(kc + 1) * 128, :])
                nc.scalar.dma_start(out=wv_sb[:, kc, :], in_=wv[kc * 128:(kc + 1) * 128, :])
            bq_sb = bk_sb = bv_sb = None
            if has_bqk:
                bq_sb = consts.tile([128, 1], F32, name="bq_sb")
                bk_sb = consts.tile([128, 1], F32, name="bk_sb")
                nc.sync.dma_start(out=bq_sb, in_=bq)
                nc.sync.dma_start(out=bk_sb, in_=bk)
            g_sb = consts.tile([128, 1], F32, name="g_sb")
            nc.sync.dma_start(out=g_sb, in_=gscale)
            if has_bv:
                bv_sb = consts.tile([128, c], F32, name="bv_sb")
                nc.sync.dma_start(out=bv_sb, in_=bv.to_broadcast([128, c]))
            xTq_sb = consts.tile([128, kc_n, rows], BF16, name="xTq_sb")
            for kc in range(kc_n):
                nc.sync.dma_start(out=xTq_sb[:, kc, :],
                                  in_=xTq[kc * 128:(kc + 1) * 128, :])

            # ---- x column-sliced, kc-interleaved so consumers start early;
            # alternate between two HWDGE queues to double stream bandwidth ----
            xT_sb = consts.tile([128, kc_n, n], BF16, name="xT_sb")
            engs = [nc.sync, nc.scalar]
            di = 0
            for o, w in _slices(n, 2048):
                for kc in range(kc_n):
                    engs[di % 2].dma_start(out=xT_sb[:, kc, o:o + w],
                                           in_=xT[kc * 128:(kc + 1) * 128, o:o + w])
                    di += 1

            # rotate PSUM->SBUF projection copies across three engines so a
            # single engine's copy throughput never paces the projections.
            # Tile remaps generic copies onto DVE, so use engine-exclusive
            # instructions: InstPool (GpSimd) and InstActivation-Identity
            # (ScalarE) cannot be remapped.
            _rot = [0]

            def proj_copy(dst, src, bias_sb):
                k = _rot[0] % 2
                _rot[0] += 1
                if has_bqk:
                    if k == 0:
                        nc.vector.tensor_scalar_add(dst, src, bias_sb)
                    else:
                        nc.scalar.activation(dst, src, AF.Identity, bias=bias_sb,
                                             scale=1.0)
                else:
                    if k == 0:
                        nc.vector.tensor_copy(out=dst, in_=src)
                    else:
                        nc.scalar.activation(dst, src, AF.Identity, scale=1.0)

            # ---- projections (qT/kT replicated over four 32-row blocks) ----
            projp = tc.alloc_tile_pool(name="projp", bufs=4, space="PSUM")
            qT_sb = consts.tile([128, rows], BF16, name="qT_sb")
            for o, w in _slices(rows, 512):
                qps = projp.tile([128, 512], F32, tag="pps", name=f"qps_{o}")
                for kc in range(kc_n):
                    nc.tensor.matmul(qps[:, :w], lhsT=wq_sb[:, kc, :],
                                     rhs=xTq_sb[:, kc, o:o + w],
                                     start=(kc == 0), stop=(kc == kc_n - 1))
                proj_copy(qT_sb[:, o:o + w], qps[:, :w], bq_sb)

            # ---- kT and v projections, interleaved per 512-column slice so
            # PE consumption tracks the streaming xT DMA (v chunk j only needs
            # xT columns up to j*128+128, so it must not wait for all of kT) ----
            kT_sb = consts.tile([128, n], BF16, name="kT_sb")
            vdt = FP8 if USE_FP8 else BF16
            vw = VPAD if USE_FP8 else c + 1
            v_sb = consts.tile([128, n_j, vw], vdt, name="v_sb")
            nc.gpsimd.memset(v_sb[:, :, c], 1.0)
            for o, w in _slices(n, 512):
                kps = projp.tile([128, 512], F32, tag="pps", name=f"kps_{o}")
                for kc in range(kc_n):
                    nc.tensor.matmul(kps[:, :w], lhsT=wk_sb[:, kc, :],
                                     rhs=xT_sb[:, kc, o:o + w],
                                     start=(kc == 0), stop=(kc == kc_n - 1))
                proj_copy(kT_sb[:, o:o + w], kps[:, :w], bk_sb)
                for jb in range(o // 128, min((o + w + 127) // 128, n_j)):
                    jo, jsz = jchunks[jb]
                    vps = projp.tile([128, 512], F32, tag="pps", name=f"vps_{jb}")
                    for kc in range(kc_n):
                        nc.tensor.matmul(vps[:jsz, :c],
                                         lhsT=xT_sb[:, kc, jo:jo + jsz],
                                         rhs=wv_sb[:, kc, :],
                                         start=(kc == 0), stop=(kc == kc_n - 1))
                    if has_bv:
                        nc.vector.tensor_add(v_sb[:jsz, jb, 0:c], vps[:jsz, :c],
                                             bv_sb[:jsz, :])
                    else:
                        if jb % 2 == 0:
                            nc.vector.tensor_copy(out=v_sb[:jsz, jb, 0:c],
                                                  in_=vps[:jsz, :c])
                        else:
                            nc.scalar.activation(v_sb[:jsz, jb, 0:c],
                                                 vps[:jsz, :c],
                                                 AF.Identity, scale=1.0)

            projp.release()

            # ---- main attention loop ----
            accp = tc.alloc_tile_pool(name="accp", bufs=1, space="PSUM")
            blocks = [groups[b:b + BLOCK_G] for b in range(0, len(groups), BLOCK_G)]
            pending_epilogue = None

            for i0, iw in _slices(rows, 512):
                subs = _slices(iw, 128)
                accs = [accp.tile([128, c + 1], F32, tag=f"acc{s}",
                                  name=f"acc_{i0}_{s}")
                        for s in range(len(subs))]

                def emit_block_st(b, i0=i0, iw=iw):
                    """Scores + exp for one block -> one SBUF se tile."""
                    bg = blocks[b]
                    seb = sep.tile([128, GROUP * BLOCK_G, 512],
                                   FP8 if USE_FP8 else BF16, tag="se",
                                   name=f"se_{i0}_{b}")
                    for gi, chunks in enumerate(bg):
                        stg = psp.tile([128, GROUP, 512], F32, tag="ps",
                                       name=f"st_{i0}_{b}_{gi}")
                        for t, jb in enumerate(chunks):
                            jo, jsz = jchunks[jb]
                            nc.tensor.matmul(stg[:jsz, t, :iw],
                                             lhsT=kT_sb[32 * t:32 * t + 32,
                                                        jo:jo + jsz],
                                             rhs=qT_sb[32 * t:32 * t + 32,
                                                       i0:i0 + iw],
                                             start=True, stop=True,
                                             tile_position=(32 * t, 0))
                        sizes = {jchunks[jb][1] for jb in chunks}
                        so_g = gi * GROUP
                        if len(sizes) == 1:
                            jsz = sizes.pop()
                            nc.scalar.activation(
                                seb[:jsz, so_g:so_g + len(chunks), :iw],
                                stg[:jsz, :len(chunks), :iw], AF.Exp, scale=scale)
                        else:
                            for t, jb in enumerate(chunks):
                                jsz = jchunks[jb][1]
                                nc.scalar.activation(
                                    seb[:jsz, so_g + t, :iw],
                                    stg[:jsz, t, :iw], AF.Exp, scale=scale)
                    return seb

                def emit_block_acc(b, seb, accs=accs, subs=subs, i0=i0):
                    """attn@v for one block: j-inner same-bank chains.

                    With USE_FP8, each full pair of key chunks is contracted by
                    one DoubleRow matmul (virtual K=256, 2 fp8 weights/cell).
                    """
                    bg = blocks[b]
                    for s, (so, sw) in enumerate(subs):
                        for gi, chunks in enumerate(bg):
                            q = gi * GROUP
                            sizes = [jchunks[jb][1] for jb in chunks]
                            if USE_FP8 and len(chunks) == 2 and sizes[0] == sizes[1]:
                                jsz = sizes[0]
                                jb0 = chunks[0]
                                nc.tensor.matmul(
                                    accs[s][:sw, :],
                                    lhsT=seb[:jsz, q:q + 2, so:so + sw],
                                    rhs=v_sb[:jsz, jb0:jb0 + 2, 0:c + 1],
                                    start=(jb0 == 0), stop=(jb0 + 1 == n_j - 1),
                                    perf_mode=mybir.MatmulPerfMode.DoubleRow)
                            else:
                                for t, jb in enumerate(chunks):
                                    jo, jsz = jchunks[jb]
                                    nc.tensor.matmul(
                                        accs[s][:sw, :],
                                        lhsT=seb[:jsz, q + t, so:so + sw],
                                        rhs=v_sb[:jsz, jb, 0:c + 1],
                                        start=(jb == 0), stop=(jb == n_j - 1))

                nb = len(blocks)
                seb_q = [emit_block_st(b) for b in range(min(2, nb))]
                if pending_epilogue is not None:
                    pending_epilogue()
                    pending_epilogue = None
                for b in range(nb):
                    if b + 2 < nb:
                        seb_q.append(emit_block_st(b + 2))
                    emit_block_acc(b, seb_q[b])

                def epilogue(i0=i0, subs=subs, accs=accs):
                    for s, (so, sw) in enumerate(subs):
                        g0 = i0 + so
                        rec = smallp.tile([128, 1], F32, tag="rec",
                                          name=f"rec_{i0}_{s}")
                        nc.vector.reciprocal(rec[:sw], accs[s][:sw, c:c + 1])
                        sc = smallp.tile([128, 1], F32, tag="sc",
                                         name=f"sc_{i0}_{s}")
                        nc.vector.tensor_mul(sc[:sw], rec[:sw], g_sb[:sw])
                        xr = xrp.tile([128, c], F32, tag="xr",
                                      name=f"xr_{i0}_{s}")
                        nc.sync.dma_start(out=xr[:sw, :], in_=xres[g0:g0 + sw, :])
                        ot = otp.tile([128, c], F32, tag="ot",
                                      name=f"ot_{i0}_{s}")
                        nc.vector.scalar_tensor_tensor(
                            out=ot[:sw, :], in0=accs[s][:sw, 0:c],
                            scalar=sc[:sw], in1=xr[:sw, :],
                            op0=ALU.mult, op1=ALU.add)
                        nc.sync.dma_start(out=out[g0:g0 + sw, :], in_=ot[:sw, :])

                pending_epilogue = epilogue

            pending_epilogue()
            accp.release()

    nc.compile()
    return nc


_BUILD_CACHE = {}


def _get_built(n, rows, c, d, has_bv, has_bqk=False):
    key = (n, rows, c, d, has_bv, has_bqk)
    if key not in _BUILD_CACHE:
        _BUILD_CACHE[key] = build(n, rows, c, d, has_bv, has_bqk)
    return _BUILD_CACHE[key]


def make_in_maps(x2, Wq, bq, Wk, bk, Wv, bv, gamma, n, rows, c, d, n_cores):
    """Host-side prep: x2 is the flattened [n, c] fp32 token matrix."""
    has_bv = bool(np.any(np.asarray(bv) != 0.0))
    has_bqk = bool(np.any(np.asarray(bq) != 0.0) or np.any(np.asarray(bk) != 0.0))
    xT32 = np.ascontiguousarray(x2.T)
    xT_bf = xT32.astype(ml_dtypes.bfloat16)
    wq_bf = np.tile(np.asarray(Wq, np.float32), (1, 4)).astype(ml_dtypes.bfloat16)
    wk_bf = np.tile(np.asarray(Wk, np.float32), (1, 4)).astype(ml_dtypes.bfloat16)
    wv_bf = np.asarray(Wv, np.float32).astype(ml_dtypes.bfloat16)
    gs = np.full((128, 1), np.float32(np.asarray(gamma).reshape(-1)[0]), np.float32)

    in_maps = []
    for cid in range(n_cores):
        r0 = cid * rows
        m = {
            "xT": xT_bf,
            "xTq": np.ascontiguousarray(xT_bf[:, r0:r0 + rows]),
            "xres": np.ascontiguousarray(x2[r0:r0 + rows]),
            "wq": wq_bf, "wk": wk_bf, "wv": wv_bf,
            "gscale": gs,
        }
        if has_bqk:
            m["bq"] = np.ascontiguousarray(
                np.tile(np.asarray(bq, np.float32), 4)[:, None])
            m["bk"] = np.ascontiguousarray(
                np.tile(np.asarray(bk, np.float32), 4)[:, None])
        if has_bv:
            m["bv"] = np.asarray(bv, np.float32).reshape(1, c)
        in_maps.append(m)
    return in_maps, (has_bv, has_bqk)


def kernel(x, Wq, bq, Wk, bk, Wv, bv, gamma):
    x = np.asarray(x, np.float32)
    B, H, W, D_, C = x.shape
    n = H * W * D_ * B
    assert (n, C) == (FULL_N, FULL_C)
    rows = n // N_CORES
    d = Wq.shape[1]

    x2 = np.ascontiguousarray(x.reshape(n, C))
    in_maps, (has_bv, has_bqk) = make_in_maps(
        x2, Wq, bq, Wk, bk, Wv, bv, gamma, n, rows, C, d, N_CORES)
    nc = _get_built(n, rows, C, d, has_bv, has_bqk)
    res = run_bass_kernel_spmd(nc, in_maps, core_ids=list(range(N_CORES)))
    full = np.concatenate([res.results[cid]["out"] for cid in range(N_CORES)], axis=0)
    return full.reshape(B, H, W, D_, C)


# revision 19
# speedup vs baseline: 1.0074x; 1.0074x over previous
"""Trainium2 Bass kernel: SAM2-style full self-attention over a 3D volume.

Computes  out = x + gamma * softmax((x Wq + bq)(x Wk + bk)^T / sqrt(d)) (x Wv + bv)
for x of shape [1, 20, 20, 20, 256]  (N = 8000 tokens, C = 256, d = 32).

Sharding: sequence-parallel over query rows. Core c owns output rows
[c*1000, (c+1)*1000). Every core recomputes the full K / V projections from
the complete x (only ~8 MB), so no collectives are needed.

Per-core dataflow (bf16 matmuls, fp32 PSUM accumulation, fp32 residual):
  - host pre-transposes x -> xT [C, N]; the channel contraction is the SBUF
    partition dim everywhere, so there are no on-device transposes.
  - qT/kT are replicated across four 32-partition blocks (host tiles Wq/Wk to
    [C, 128]) so pairs of K=32 score matmuls pack into disjoint PE row groups
    (tile_position) and run concurrently.
  - v [N, 257]: col 256 memset to 1.0 - the ones column makes attn@v also
    produce the softmax denominators for free.
  - scores are computed in groups of two 128-key chunks into one 2-bank PSUM
    tile; a single Exp activation per group (scale 1/sqrt(d) folded in; no
    max-subtraction needed: |scores|/sqrt(d) < ~6 so fp32 exp cannot
    overflow) writes bf16 S_exp^T into a per-block SBUF buffer.
  - attn@v runs j-inner per 128-query sub-tile: long same-PSUM-bank
    accumulation chains, K=128, N=257.
  - epilogue: out = (attended[:, :256] * gamma / attended[:, 256]) + x.
"""

import math

import numpy as np
import ml_dtypes

import concourse.bacc as bacc
import concourse.mybir as mybir
import concourse.tile as tile
from concourse.bass_utils import run_bass_kernel_spmd

BF16 = mybir.dt.bfloat16
F32 = mybir.dt.float32
AF = mybir.ActivationFunctionType
ALU = mybir.AluOpType

N_CORES = 8
FULL_N = 8000
FULL_C = 256

GROUP = 2     # score j-chunks per PSUM tile / Exp call (row-group packing)
BLOCK_G = 4   # groups per attn block (8 j-chunks): same-bank chain length
USE_FP8 = False  # fp8e4m3 S_exp/v with DoubleRow attn@v (2 key chunks per matmul)
FP8 = mybir.dt.float8e4
VPAD = 272    # v chunk stride (16B-aligned for DoubleRow APs)


def _slices(total, width):
    out = []
    o = 0
    while o < total:
        w = min(width, total - o)
        out.append((o, w))
        o += w
    return out


def build(n, rows, c, d, has_bv, has_bqk):
    assert c % 128 == 0
    assert 4 * d == 128
    kc_n = c // 128
    jchunks = _slices(n, 128)
    n_j = len(jchunks)
    groups = [list(range(g, min(g + GROUP, n_j))) for g in range(0, n_j, GROUP)]
    scale = 1.0 / math.sqrt(d)

    nc = bacc.Bacc("TRN2", target_bir_lowering=False, debug=False,
                   enable_asserts=False, num_devices=N_CORES)

    xT = nc.dram_tensor("xT", [c, n], BF16, kind="ExternalInput").ap()
    xTq = nc.dram_tensor("xTq", [c, rows], BF16, kind="ExternalInput").ap()
    xres = nc.dram_tensor("xres", [rows, c], F32, kind="ExternalInput").ap()
    wq = nc.dram_tensor("wq", [c, 4 * d], BF16, kind="ExternalInput").ap()
    wk = nc.dram_tensor("wk", [c, 4 * d], BF16, kind="ExternalInput").ap()
    wv = nc.dram_tensor("wv", [c, c], BF16, kind="ExternalInput").ap()
    bq = bk = bv = None
    if has_bqk:
        bq = nc.dram_tensor("bq", [128, 1], F32, kind="ExternalInput").ap()
        bk = nc.dram_tensor("bk", [128, 1], F32, kind="ExternalInput").ap()
    if has_bv:
        bv = nc.dram_tensor("bv", [1, c], F32, kind="ExternalInput").ap()
    gscale = nc.dram_tensor("gscale", [128, 1], F32, kind="ExternalInput").ap()
    out = nc.dram_tensor("out", [rows, c], F32, kind="ExternalOutput").ap()

    with tile.TileContext(nc) as tc:
        with (
            tc.tile_pool(name="consts", bufs=1) as consts,
            tc.tile_pool(name="sexp", bufs=3) as sep,
            tc.tile_pool(name="small", bufs=4) as smallp,
            tc.tile_pool(name="xrp", bufs=3) as xrp,
            tc.tile_pool(name="otp", bufs=3) as otp,
            tc.tile_pool(name="psp", bufs=2, space="PSUM") as psp,
        ):
            # ---- small inputs first: weights, biases, per-core query cols ----
            wq_sb = consts.tile([128, kc_n, 4 * d], BF16, name="wq_sb")
            wk_sb = consts.tile([128, kc_n, 4 * d], BF16, name="wk_sb")
            wv_sb = consts.tile([128, kc_n, c], BF16, name="wv_sb")
            for kc in range(kc_n):
                nc.scalar.dma_start(out=wq_sb[:, kc, :], in_=wq[kc * 128:(kc + 1) * 128, :])
                nc.scalar.dma_start(out=wk_sb[:, kc, :], in_=wk[kc * 128:(kc + 1) * 128, :])
                nc.scalar.dma_start(out=wv_sb[:, kc, :], in_=wv[kc * 128:(kc + 1) * 128, :])
            bq_sb = bk_sb = bv_sb = None
            if has_bqk:
                bq_sb = consts.tile([128, 1], F32, name="bq_sb")
                bk_sb = consts.tile([128, 1], F32, name="bk_sb")
                nc.sync.dma_start(out=bq_sb, in_=bq)
                nc.sync.dma_start(out=bk_sb, in_=bk)
            g_sb = consts.tile([128, 1], F32, name="g_sb")
            nc.sync.dma_start(out=g_sb, in_=gscale)
            if has_bv:
                bv_sb = consts.tile([128, c], F32, name="bv_sb")
                nc.sync.dma_start(out=bv_sb, in_=bv.to_broadcast([128, c]))
            xTq_sb = consts.tile([128, kc_n, rows], BF16, name="xTq_sb")
            for kc in range(kc_n):
                nc.sync.dma_start(out=xTq_sb[:, kc, :],
                                  in_=xTq[kc * 128:(kc + 1) * 128, :])

            # ---- x column-sliced, kc-interleaved so consumers start early;
            # alternate between two HWDGE queues to double stream bandwidth ----
            xT_sb = consts.tile([128, kc_n, n], BF16, name="xT_sb")
            engs = [nc.sync, nc.scalar]
            di = 0
            for o, w in _slices(n, 2048):
                for kc in range(kc_n):
                    engs[di % 2].dma_start(out=xT_sb[:, kc, o:o + w],
                                           in_=xT[kc * 128:(kc + 1) * 128, o:o + w])
                    di += 1

            # rotate PSUM->SBUF projection copies across three engines so a
            # single engine's copy throughput never paces the projections.
            # Tile remaps generic copies onto DVE, so use engine-exclusive
            # instructions: InstPool (GpSimd) and InstActivation-Identity
            # (ScalarE) cannot be remapped.
            _rot = [0]

            def proj_copy(dst, src, bias_sb):
                k = _rot[0] % 2
                _rot[0] += 1
                if has_bqk:
                    if k == 0:
                        nc.vector.tensor_scalar_add(dst, src, bias_sb)
                    else:
                        nc.scalar.activation(dst, src, AF.Identity, bias=bias_sb,
                                             scale=1.0)
                else:
                    if k == 0:
                        nc.vector.tensor_copy(out=dst, in_=src)
                    else:
                        nc.scalar.activation(dst, src, AF.Identity, scale=1.0)

            # ---- projections (qT/kT replicated over four 32-row blocks) ----
            projp = tc.alloc_tile_pool(name="projp", bufs=4, space="PSUM")
            qT_sb = consts.tile([128, rows], BF16, name="qT_sb")
            for o, w in _slices(rows, 512):
                qps = projp.tile([128, 512], F32, tag="pps", name=f"qps_{o}")
                for kc in range(kc_n):
                    nc.tensor.matmul(qps[:, :w], lhsT=wq_sb[:, kc, :],
                                     rhs=xTq_sb[:, kc, o:o + w],
                                     start=(kc == 0), stop=(kc == kc_n - 1))
                proj_copy(qT_sb[:, o:o + w], qps[:, :w], bq_sb)

            # ---- kT and v projections, interleaved per 512-column slice so
            # PE consumption tracks the streaming xT DMA (v chunk j only needs
            # xT columns up to j*128+128, so it must not wait for all of kT) ----
            kT_sb = consts.tile([128, n], BF16, name="kT_sb")
            vdt = FP8 if USE_FP8 else BF16
            vw = VPAD if USE_FP8 else c + 1
            v_sb = consts.tile([128, n_j, vw], vdt, name="v_sb")
            nc.gpsimd.memset(v_sb[:, :, c], 1.0)
            for o, w in _slices(n, 512):
                kps = projp.tile([128, 512], F32, tag="pps", name=f"kps_{o}")
                for kc in range(kc_n):
                    nc.tensor.matmul(kps[:, :w], lhsT=wk_sb[:, kc, :],
                                     rhs=xT_sb[:, kc, o:o + w],
                                     start=(kc == 0), stop=(kc == kc_n - 1))
                proj_copy(kT_sb[:, o:o + w], kps[:, :w], bk_sb)
                for jb in range(o // 128, min((o + w + 127) // 128, n_j)):
                    jo, jsz = jchunks[jb]
                    vps = projp.tile([128, 512], F32, tag="pps", name=f"vps_{jb}")
                    for kc in range(kc_n):
                        nc.tensor.matmul(vps[:jsz, :c],
                                         lhsT=xT_sb[:, kc, jo:jo + jsz],
                                         rhs=wv_sb[:, kc, :],
                                         start=(kc == 0), stop=(kc == kc_n - 1))
                    if has_bv:
                        nc.vector.tensor_add(v_sb[:jsz, jb, 0:c], vps[:jsz, :c],
                                             bv_sb[:jsz, :])
                    else:
                        if jb % 2 == 0:
                            nc.vector.tensor_copy(out=v_sb[:jsz, jb, 0:c],
                                                  in_=vps[:jsz, :c])
                        else:
                            nc.scalar.activation(v_sb[:jsz, jb, 0:c],
                                                 vps[:jsz, :c],
                                                 AF.Identity, scale=1.0)

            projp.release()

            # ---- main attention loop ----
            accp = tc.alloc_tile_pool(name="accp", bufs=1, space="PSUM")
            blocks = [groups[b:b + BLOCK_G] for b in range(0, len(groups), BLOCK_G)]
            pending_epilogue = None

            for i0, iw in _slices(rows, 512):
                subs = _slices(iw, 128)
                accs = [accp.tile([128, c + 1], F32, tag=f"acc{s}",
                                  name=f"acc_{i0}_{s}")
                        for s in range(len(subs))]

                def emit_block_st(b, i0=i0, iw=iw):
                    """Scores + exp for one block -> one SBUF se tile."""
                    bg = blocks[b]
                    seb = sep.tile([128, GROUP * BLOCK_G, 512],
                                   FP8 if USE_FP8 else BF16, tag="se",
                                   name=f"se_{i0}_{b}")
                    for gi, chunks in enumerate(bg):
                        stg = psp.tile([128, GROUP, 512], F32, tag="ps",
                                       name=f"st_{i0}_{b}_{gi}")
                        for t, jb in enumerate(chunks):
                            jo, jsz = jchunks[jb]
                            nc.tensor.matmul(stg[:jsz, t, :iw],
                                             lhsT=kT_sb[32 * t:32 * t + 32,
                                                        jo:jo + jsz],
                                             rhs=qT_sb[32 * t:32 * t + 32,
                                                       i0:i0 + iw],
                                             start=True, stop=True,
                                             tile_position=(32 * t, 0))
                        sizes = {jchunks[jb][1] for jb in chunks}
                        so_g = gi * GROUP
                        if len(sizes) == 1:
                            jsz = sizes.pop()
                            nc.scalar.activation(
                                seb[:jsz, so_g:so_g + len(chunks), :iw],
                                stg[:jsz, :len(chunks), :iw], AF.Exp, scale=scale)
                        else:
                            for t, jb in enumerate(chunks):
                                jsz = jchunks[jb][1]
                                nc.scalar.activation(
                                    seb[:jsz, so_g + t, :iw],
                                    stg[:jsz, t, :iw], AF.Exp, scale=scale)
                    return seb

                def emit_block_acc(b, seb, accs=accs, subs=subs, i0=i0):
                    """attn@v for one block: j-inner same-bank chains.

                    With USE_FP8, each full pair of key chunks is contracted by
                    one DoubleRow matmul (virtual K=256, 2 fp8 weights/cell).
                    """
                    bg = blocks[b]
                    for s, (so, sw) in enumerate(subs):
                        for gi, chunks in enumerate(bg):
                            q = gi * GROUP
                            sizes = [jchunks[jb][1] for jb in chunks]
                            if USE_FP8 and len(chunks) == 2 and sizes[0] == sizes[1]:
                                jsz = sizes[0]
                                jb0 = chunks[0]
                                nc.tensor.matmul(
                                    accs[s][:sw, :],
                                    lhsT=seb[:jsz, q:q + 2, so:so + sw],
                                    rhs=v_sb[:jsz, jb0:jb0 + 2, 0:c + 1],
                                    start=(jb0 == 0), stop=(jb0 + 1 == n_j - 1),
                                    perf_mode=mybir.MatmulPerfMode.DoubleRow)
                            else:
                                for t, jb in enumerate(chunks):
                                    jo, jsz = jchunks[jb]
                                    nc.tensor.matmul(
                                        accs[s][:sw, :],
                                        lhsT=seb[:jsz, q + t, so:so + sw],
                                        rhs=v_sb[:jsz, jb, 0:c + 1],
                                        start=(jb == 0), stop=(jb == n_j - 1))

                nb = len(blocks)
                seb_q = [emit_block_st(b) for b in range(min(2, nb))]
                if pending_epilogue is not None:
                    pending_epilogue()
                    pending_epilogue = None
                for b in range(nb):
                    if b + 2 < nb:
                        seb_q.append(emit_block_st(b + 2))
                    emit_block_acc(b, seb_q[b])

                def epilogue(i0=i0, subs=subs, accs=accs):
                    for s, (so, sw) in enumerate(subs):
                        g0 = i0 + so
                        rec = smallp.tile([128, 1], F32, tag="rec",
                                          name=f"rec_{i0}_{s}")
                        nc.vector.reciprocal(rec[:sw], accs[s][:sw, c:c + 1])
                        sc = smallp.tile([128, 1], F32, tag="sc",
                                         name=f"sc_{i0}_{s}")
                        nc.vector.tensor_mul(sc[:sw], rec[:sw], g_sb[:sw])
                        xr = xrp.tile([128, c], F32, tag="xr",
                                      name=f"xr_{i0}_{s}")
                        nc.sync.dma_start(out=xr[:sw, :], in_=xres[g0:g0 + sw, :])
                        ot = otp.tile([128, c], F32, tag="ot",
                                      name=f"ot_{i0}_{s}")
                        nc.vector.scalar_tensor_tensor(
                            out=ot[:sw, :], in0=accs[s][:sw, 0:c],
                            scalar=sc[:sw], in1=xr[:sw, :],
                            op0=ALU.mult, op1=ALU.add)
                        nc.sync.dma_start(out=out[g0:g0 + sw, :], in_=ot[:sw, :])

                pending_epilogue = epilogue

            pending_epilogue()
            accp.release()

    nc.compile()
    return nc


_BUILD_CACHE = {}


def _get_built(n, rows, c, d, has_bv, has_bqk=False):
    key = (n, rows, c, d, has_bv, has_bqk)
    if key not in _BUILD_CACHE:
        _BUILD_CACHE[key] = build(n, rows, c, d, has_bv, has_bqk)
    return _BUILD_CACHE[key]


def make_in_maps(x2, Wq, bq, Wk, bk, Wv, bv, gamma, n, rows, c, d, n_cores):
    """Host-side prep: x2 is the flattened [n, c] fp32 token matrix."""
    has_bv = bool(np.any(np.asarray(bv) != 0.0))
    has_bqk = bool(np.any(np.asarray(bq) != 0.0) or np.any(np.asarray(bk) != 0.0))
    xT32 = np.ascontiguousarray(x2.T)
    xT_bf = xT32.astype(ml_dtypes.bfloat16)
    wq_bf = np.tile(np.asarray(Wq, np.float32), (1, 4)).astype(ml_dtypes.bfloat16)
    wk_bf = np.tile(np.asarray(Wk, np.float32), (1, 4)).astype(ml_dtypes.bfloat16)
    wv_bf = np.asarray(Wv, np.float32).astype(ml_dtypes.bfloat16)
    gs = np.full((128, 1), np.float32(np.asarray(gamma).reshape(-1)[0]), np.float32)

    in_maps = []
    for cid in range(n_cores):
        r0 = cid * rows
        m = {
            "xT": xT_bf,
            "xTq": np.ascontiguousarray(xT_bf[:, r0:r0 + rows]),
            "xres": np.ascontiguousarray(x2[r0:r0 + rows]),
            "wq": wq_bf, "wk": wk_bf, "wv": wv_bf,
            "gscale": gs,
        }
        if has_bqk:
            m["bq"] = np.ascontiguousarray(
                np.tile(np.asarray(bq, np.float32), 4)[:, None])
            m["bk"] = np.ascontiguousarray(
                np.tile(np.asarray(bk, np.float32), 4)[:, None])
        if has_bv:
            m["bv"] = np.asarray(bv, np.float32).reshape(1, c)
        in_maps.append(m)
    return in_maps, (has_bv, has_bqk)


def kernel(x, Wq, bq, Wk, bk, Wv, bv, gamma):
    x = np.asarray(x, np.float32)
    B, H, W, D_, C = x.shape
    n = H * W * D_ * B
    assert (n, C) == (FULL_N, FULL_C)
    rows = n // N_CORES
    d = Wq.shape[1]

    x2 = np.ascontiguousarray(x.reshape(n, C))
    in_maps, (has_bv, has_bqk) = make_in_maps(
        x2, Wq, bq, Wk, bk, Wv, bv, gamma, n, rows, C, d, N_CORES)
    nc = _get_built(n, rows, C, d, has_bv, has_bqk)
    res = run_bass_kernel_spmd(nc, in_maps, core_ids=list(range(N_CORES)))
    full = np.concatenate([res.results[cid]["out"] for cid in range(N_CORES)], axis=0)
    return full.reshape(B, H, W, D_, C)
